# revision 4
# baseline (speedup 1.0000x reference)
"""Trainium2 Bass kernel for nn_DifferentiableLattice (gnn_message_passing).

Reference computation (per step, 9 steps):
    m = max(state)                         # global over (B, N)
    state = state @ P.T
    state = state * angle_factor * decay
    state = sigmoid(2*state - 1) * max(m, 0.1)
then out = sum_t softmax(step_weights)[t] * state_t   (incl. state_0 = x)

Kernel strategy (8 NeuronCores, data-parallel over batch):
  * Host precomputes W2 = 2*decay*diag(angle_factor) @ P (512x512, bf16), the
    softmax weights w[t], and ships each core's batch shard ALREADY TRANSPOSED
    (x^T, bf16 [512 cells, 2048 batch]); the core returns its accumulator
    transposed ([512, 2048] f32) and the host transposes back.  This removes
    all on-chip PE transposes / PSUM-copy traffic from the old design.
  * On-chip state is the unscaled sigmoid output s~_t in bf16, kept
    [cells(part), batch(free)]:
        raw_t  = W2 @ s~_{t-1}      TensorE bf16, f32 psum [128,2048] tiles,
                                    k-outer loop so next step's k-th matmuls
                                    only need this step's j=k ACT output
        s~_t   = sigmoid(c_{t-1} * raw_t - 1)    ScalarE, writes bf16 st
        q_t    = coef_t * s~_t                    VectorE tensor_scalar (4x
                 + pmt = max(q_t) per partition   bf16 mode), accum_out=max
        acc   += q_t                              VectorE tensor_tensor, f32
    coef_t = w_t * c_t.  Because pmt is the max of coef_t*s~_t, the
    cross-device AllReduce(max) result G'_t = w_t*c_t*gmax_t gives
    c_{t+1} = max(G'_t / w_t, 0.1) with 1/w_t a compile-time constant, so
    the c-recurrence depends only on the collective result (2-step slack).
  * The very first AllReduce (max of x) launches right after the x^T DMAs
    land, absorbing cross-core startup skew during step 1's matmuls.
"""

import os
import sys

import numpy as np

sys.path.insert(0, "/opt/trn_rl_repo")

from contextlib import ExitStack

import concourse.bacc as bacc
import concourse.bass as bass
import concourse.bass_isa as bass_isa
import concourse.mybir as mybir
import concourse.tile as tile
from concourse.bass_utils import run_bass_kernel_spmd

F32 = mybir.dt.float32
BF16 = mybir.dt.bfloat16
ALU = mybir.AluOpType
AX = mybir.AxisListType
ACTF = mybir.ActivationFunctionType

N_CELLS = 512
BATCH = 16384
N_CORES = 8
BSH = BATCH // N_CORES          # 2048 batch rows per core
KT = N_CELLS // 128             # 4 cell partition-tiles
NB = BSH // 512                 # 4 batch chunks of 512 (matmul moving max)

LAST_RESULTS = None             # test harness peeks at this for profiling


def _host_prep(adjacency, std_devs, split_probs, join_probs, bounce_angles,
               step_weights, decay_rate, n_steps):
    """Replicate the reference's parameter preprocessing in float64."""
    adjacency = np.asarray(adjacency, np.float64)
    std_devs = np.asarray(std_devs, np.float64)
    split_probs = np.asarray(split_probs, np.float64)
    join_probs = np.asarray(join_probs, np.float64)
    step_weights = np.asarray(step_weights, np.float64)
    decay_rate = np.asarray(decay_rate, np.float64)

    max_steps = step_weights.shape[0]
    actual_steps = min(int(n_steps), max_steps)
    # torch.clamp(x, min=2.0, max=0.99) saturates at 0.99
    decay = float(np.minimum(np.maximum(decay_rate, 2.0), 0.99)[0])

    from scipy.special import erf
    threshold = 0.5
    s = np.maximum(np.abs(std_devs), 2.0)
    straight = erf(threshold / (s * np.sqrt(2.0)))
    sp = np.clip(split_probs, 0.0, 1.0)
    jp = np.clip(join_probs, 0.0, 1.0)
    self_retention = straight * 0.3 * (1.0 - sp * 0.5)
    spread_factor = (1.0 - straight + sp * 0.3)[:, None]
    join_boost = (1.0 + jp * 0.5)[None, :]
    neighbor_spread = adjacency * spread_factor * join_boost
    prop = np.diag(self_retention) + neighbor_spread * 0.7
    prop = prop / np.clip(prop.sum(axis=1, keepdims=True), 1e-6, None)

    ang = np.clip(np.asarray(bounce_angles, np.float64), 0.0, 2.0)
    angle_factor = 0.5 + 0.5 * np.cos(ang.mean(axis=1))

    W2 = (2.0 * decay) * (angle_factor[:, None] * prop)     # (N, N) rows j
    sw = step_weights[: actual_steps + 1]
    sw = sw - sw.max()
    e = np.exp(sw)
    w = e / e.sum()                                          # softmax weights

    return actual_steps, np.ascontiguousarray(W2.T), w.astype(np.float64)


def _build_program(steps, w):
    """Emit the SPMD Tile program for `steps` propagation steps.

    w: numpy float array of length steps+1 (softmax history weights).
    """
    nc = bacc.Bacc("TRN2", target_bir_lowering=False, debug=False,
                   num_devices=N_CORES)

    xt_d = nc.dram_tensor("xt", [N_CELLS, BSH], BF16, kind="ExternalInput")
    w2t_d = nc.dram_tensor("w2t", [N_CELLS, N_CELLS], BF16, kind="ExternalInput")
    out_d = nc.dram_tensor("out", [N_CELLS, BSH], F32, kind="ExternalOutput")

    groups = [list(range(N_CORES))]

    with tile.TileContext(nc) as tc, ExitStack() as ctx:
        const = ctx.enter_context(tc.tile_pool(name="const", bufs=1))
        qp = ctx.enter_context(tc.tile_pool(name="qp", bufs=6))
        small = ctx.enter_context(tc.tile_pool(name="small", bufs=3))
        psp = ctx.enter_context(tc.tile_pool(name="psp", bufs=2, space="PSUM"))
        ccd = ctx.enter_context(tc.tile_pool(name="ccd", bufs=3, space="DRAM"))

        neg1 = const.tile([128, 1], F32, tag="neg1", name="neg1")
        nc.vector.memset(neg1[:], -1.0)

        w2t = [const.tile([128, N_CELLS], BF16, tag=f"w2t{k}", name=f"w2t{k}")
               for k in range(KT)]
        for k in range(KT):
            nc.sync.dma_start(w2t[k][:], w2t_d[k * 128:(k + 1) * 128, :])

        # double-buffered transposed state s~ [cell(part), batch(free)], bf16
        st = [[const.tile([128, BSH], BF16, tag=f"st{p}{k}", name=f"st{p}{k}")
               for k in range(KT)] for p in range(2)]
        acc = [const.tile([128, BSH], F32, tag=f"acc{j}", name=f"acc{j}")
               for j in range(KT)]

        # ---------------- prologue: x^T arrives pre-transposed from the host
        for k in range(KT):
            nc.sync.dma_start(st[0][k][:], xt_d[k * 128:(k + 1) * 128, :])

        # local per-partition max of state_0 = x; launch AllReduce ASAP so the
        # first collective (incl. cross-core sync skew) hides under step 1
        pmt = small.tile([128, KT], F32, tag="pmt", name="pmt")
        for k in range(KT):
            scr = qp.tile([128, BSH], BF16, tag="q", name="q")
            nc.vector.tensor_scalar(scr[:], st[0][k][:], 1.0, None,
                                    op0=ALU.mult, op1=ALU.max,
                                    accum_out=pmt[:, k:k + 1])

        def launch_allreduce(pmt_tile):
            pm = small.tile([128, 1], F32, tag="pm", name="pm")
            nc.vector.reduce_max(pm[:], pmt_tile[:], axis=AX.X)
            pmr = small.tile([128, 1], F32, tag="pmr", name="pmr")
            nc.gpsimd.partition_all_reduce(pmr[:], pm[:], channels=128,
                                           reduce_op=bass_isa.ReduceOp.max)
            cin = small.tile([1, 8], F32, tag="cin", name="cin")
            nc.vector.memset(cin[:], 0.0)
            nc.vector.tensor_copy(cin[0:1, 0:1], pmr[0:1, 0:1])
            cc_in = ccd.tile([1, 8], F32, tag="ccin", name="ccin")
            cc_out = ccd.tile([1, 8], F32, tag="ccout", name="ccout")
            nc.gpsimd.dma_start(cc_in[:], cin[:])
            nc.gpsimd.collective_compute(
                "AllReduce", ALU.max, replica_groups=groups,
                ins=[cc_in.opt()], outs=[cc_out.opt()],
            )
            gm = small.tile([1, 8], F32, tag="gm", name="gm")
            nc.gpsimd.dma_start(gm[:], cc_out[:])
            return gm

        gm_prev = launch_allreduce(pmt)     # G'_0 = gmax(state_0)

        # acc init on ScalarE (idle during prologue): acc_j = w0 * x^T_j
        for j in range(KT):
            nc.scalar.mul(acc[j][:], st[0][j][:], float(w[0]))

        cvec_prev = None                    # c_0 == 1.0 (imm scale at t=1)

        # ---------------- main steps
        for t in range(1, steps + 1):
            ph, prev = t % 2, (t - 1) % 2

            # consume G'_{t-1}: c_t = max(G'_{t-1}/w_{t-1}, 0.1) (w_0-fold
            # skipped for the prologue max, so invw=1 at t=1)
            gmb = small.tile([128, 1], F32, tag="gmb", name="gmb")
            nc.gpsimd.partition_broadcast(gmb[:], gm_prev[0:1, 0:1],
                                          channels=128)
            invw = 1.0 if t == 1 else float(1.0 / w[t - 1])
            cvec = small.tile([128, 1], F32, tag="cvec", name="cvec", bufs=4)
            nc.vector.tensor_scalar(cvec[:], gmb[:], invw, 0.1,
                                    op0=ALU.mult, op1=ALU.max)
            coef = small.tile([128, 1], F32, tag="coef", name="coef", bufs=4)
            nc.vector.tensor_scalar(coef[:], cvec[:], float(w[t]), None,
                                    op0=ALU.mult)

            act_scale = cvec_prev           # c_{t-1}; None for t=1

            pmt = (small.tile([128, KT], F32, tag="pmt", name="pmt")
                   if t < steps else None)
            for j in range(KT):
                ps = psp.tile([128, BSH], F32, tag="ps", name="ps")
                for k in range(KT):
                    for b in range(NB):
                        nc.tensor.matmul(
                            ps[:, b * 512:(b + 1) * 512],
                            w2t[k][:, j * 128:(j + 1) * 128],
                            st[prev][k][:, b * 512:(b + 1) * 512],
                            start=(k == 0), stop=(k == KT - 1),
                        )
                nc.scalar.activation(
                    st[ph][j][:], ps[:], ACTF.Sigmoid,
                    bias=neg1[:, 0:1],
                    scale=(act_scale[:, 0:1] if act_scale is not None else 1.0),
                )
                if pmt is not None:
                    # q = coef_t*s~; per-partition max rides accum_out
                    q = qp.tile([128, BSH], BF16, tag="q", name="q")
                    nc.vector.tensor_scalar(q[:], st[ph][j][:], coef[:, 0:1],
                                            None, op0=ALU.mult, op1=ALU.max,
                                            accum_out=pmt[:, j:j + 1])
                    nc.vector.tensor_tensor(acc[j][:], acc[j][:], q[:],
                                            op=ALU.add)
                else:
                    # last step: fused accumulate, then stream the result out
                    nc.vector.scalar_tensor_tensor(
                        acc[j][:], st[ph][j][:], coef[:, 0:1], acc[j][:],
                        op0=ALU.mult, op1=ALU.add,
                    )
                    nc.sync.dma_start(out_d[j * 128:(j + 1) * 128, :],
                                      acc[j][:])

            if pmt is not None:
                gm_prev = launch_allreduce(pmt)
            cvec_prev = cvec

    nc.compile()
    return nc


def kernel(initial_activations, adjacency, std_devs, split_probs, join_probs,
           bounce_angles, step_weights, decay_rate, n_steps):
    global LAST_RESULTS
    x = np.ascontiguousarray(np.asarray(initial_activations, np.float32))
    steps, w2t_np, w = _host_prep(adjacency, std_devs, split_probs, join_probs,
                                  bounce_angles, step_weights, decay_rate,
                                  n_steps)
    if steps == 0:
        return (x * np.float32(1.0)).astype(np.float32)

    nc = _build_program(steps, w)

    bf16 = mybir.dt.np(BF16)
    w2tb = w2t_np.astype(np.float32).astype(bf16)
    in_maps = [
        {"xt": np.ascontiguousarray(x[c * BSH:(c + 1) * BSH].T).astype(bf16),
         "w2t": w2tb}
        for c in range(N_CORES)
    ]
    res = run_bass_kernel_spmd(
        nc, in_maps, core_ids=list(range(N_CORES)),
        trace=bool(os.environ.get("BASS_TRACE")),
    )
    LAST_RESULTS = res
    out = np.concatenate(
        [np.asarray(res.results[c]["out"], np.float32).T for c in range(N_CORES)],
        axis=0)
    return np.ascontiguousarray(out)


if __name__ == "__main__":
    rng = np.random.default_rng(0)
    ins = {
        "initial_activations": rng.random((BATCH, N_CELLS), np.float32),
        "adjacency": (rng.random((N_CELLS, N_CELLS)) < 6.0 / 512).astype(np.float32),
        "std_devs": rng.standard_normal(N_CELLS).astype(np.float32),
        "split_probs": rng.random(N_CELLS).astype(np.float32),
        "join_probs": rng.random(N_CELLS).astype(np.float32),
        "bounce_angles": (rng.random((N_CELLS, 6)) * 2).astype(np.float32),
        "step_weights": rng.standard_normal(10).astype(np.float32),
        "decay_rate": np.ones(1, np.float32),
        "n_steps": 9,
    }
    o = kernel(**ins)
    print("out", o.shape, o.dtype, float(o.mean()))


# revision 7
# speedup vs baseline: 1.4193x; 1.4193x over previous
"""Trainium2 Bass kernel for nn_DifferentiableLattice (gnn_message_passing).

Reference computation (per step, 9 steps):
    m = max(state)                         # global over (B, N)
    state = state @ P.T
    state = state * angle_factor * decay
    state = sigmoid(2*state - 1) * max(m, 0.1)
then out = sum_t softmax(step_weights)[t] * state_t   (incl. state_0 = x)

Kernel strategy (8 NeuronCores, data-parallel over batch):
  * Host precomputes W2 = 2*decay*diag(angle_factor) @ P (512x512, bf16), the
    softmax weights w[t], and ships each core's batch shard ALREADY TRANSPOSED
    (x^T, bf16 [512 cells, 2048 batch]); the core returns its accumulator
    transposed ([512, 2048] f32) and the host transposes back.  This removes
    all on-chip PE transposes / PSUM-copy traffic from the old design.
  * On-chip state is the unscaled sigmoid output s~_t in bf16, kept
    [cells(part), batch(free)]:
        raw_t  = W2 @ s~_{t-1}      TensorE bf16, f32 psum [128,2048] tiles,
                                    k-outer loop so next step's k-th matmuls
                                    only need this step's j=k ACT output
        s~_t   = sigmoid(c_{t-1} * raw_t - 1)    ScalarE, writes bf16 st
        q_t    = coef_t * s~_t                    VectorE tensor_scalar (4x
                 + pmt = max(q_t) per partition   bf16 mode), accum_out=max
        acc   += q_t                              VectorE tensor_tensor, f32
    coef_t = w_t * c_t.  Because pmt is the max of coef_t*s~_t, the
    cross-device AllReduce(max) result G'_t = w_t*c_t*gmax_t gives
    c_{t+1} = max(G'_t / w_t, 0.1) with 1/w_t a compile-time constant, so
    the c-recurrence depends only on the collective result (2-step slack).
  * The very first AllReduce (max of x) launches right after the x^T DMAs
    land, absorbing cross-core startup skew during step 1's matmuls.
"""

import os
import sys

import numpy as np

sys.path.insert(0, "/opt/trn_rl_repo")

from contextlib import ExitStack

import concourse.bacc as bacc
import concourse.bass as bass
import concourse.bass_isa as bass_isa
import concourse.mybir as mybir
import concourse.tile as tile
from concourse.bass_utils import run_bass_kernel_spmd

F32 = mybir.dt.float32
BF16 = mybir.dt.bfloat16
ALU = mybir.AluOpType
AX = mybir.AxisListType
ACTF = mybir.ActivationFunctionType

N_CELLS = 512
BATCH = 16384
N_CORES = 8
BSH = BATCH // N_CORES          # 2048 batch rows per core
KT = N_CELLS // 128             # 4 cell partition-tiles
NB = BSH // 512                 # 4 batch chunks of 512 (matmul moving max)

LAST_RESULTS = None             # test harness peeks at this for profiling


def _host_prep(adjacency, std_devs, split_probs, join_probs, bounce_angles,
               step_weights, decay_rate, n_steps):
    """Replicate the reference's parameter preprocessing in float64."""
    adjacency = np.asarray(adjacency, np.float64)
    std_devs = np.asarray(std_devs, np.float64)
    split_probs = np.asarray(split_probs, np.float64)
    join_probs = np.asarray(join_probs, np.float64)
    step_weights = np.asarray(step_weights, np.float64)
    decay_rate = np.asarray(decay_rate, np.float64)

    max_steps = step_weights.shape[0]
    actual_steps = min(int(n_steps), max_steps)
    # torch.clamp(x, min=2.0, max=0.99) saturates at 0.99
    decay = float(np.minimum(np.maximum(decay_rate, 2.0), 0.99)[0])

    from scipy.special import erf
    threshold = 0.5
    s = np.maximum(np.abs(std_devs), 2.0)
    straight = erf(threshold / (s * np.sqrt(2.0)))
    sp = np.clip(split_probs, 0.0, 1.0)
    jp = np.clip(join_probs, 0.0, 1.0)
    self_retention = straight * 0.3 * (1.0 - sp * 0.5)
    spread_factor = (1.0 - straight + sp * 0.3)[:, None]
    join_boost = (1.0 + jp * 0.5)[None, :]
    neighbor_spread = adjacency * spread_factor * join_boost
    prop = np.diag(self_retention) + neighbor_spread * 0.7
    prop = prop / np.clip(prop.sum(axis=1, keepdims=True), 1e-6, None)

    ang = np.clip(np.asarray(bounce_angles, np.float64), 0.0, 2.0)
    angle_factor = 0.5 + 0.5 * np.cos(ang.mean(axis=1))

    W2 = (2.0 * decay) * (angle_factor[:, None] * prop)     # (N, N) rows j
    sw = step_weights[: actual_steps + 1]
    sw = sw - sw.max()
    e = np.exp(sw)
    w = e / e.sum()                                          # softmax weights

    return actual_steps, np.ascontiguousarray(W2.T), w.astype(np.float64)


def _build_program(steps, w):
    """Emit the SPMD Tile program for `steps` propagation steps.

    w: numpy float array of length steps+1 (softmax history weights).
    """
    nc = bacc.Bacc("TRN2", target_bir_lowering=False, debug=False,
                   num_devices=N_CORES)

    xt_d = nc.dram_tensor("xt", [N_CELLS, BSH], BF16, kind="ExternalInput")
    w2t_d = nc.dram_tensor("w2t", [N_CELLS, N_CELLS], BF16, kind="ExternalInput")
    out_d = nc.dram_tensor("out", [N_CELLS, BSH], F32, kind="ExternalOutput")

    groups = [list(range(N_CORES))]

    with tile.TileContext(nc) as tc, ExitStack() as ctx:
        const = ctx.enter_context(tc.tile_pool(name="const", bufs=1))
        qp = ctx.enter_context(tc.tile_pool(name="qp", bufs=6))
        small = ctx.enter_context(tc.tile_pool(name="small", bufs=3))
        psp = ctx.enter_context(tc.tile_pool(name="psp", bufs=2, space="PSUM"))
        ccd = ctx.enter_context(tc.tile_pool(name="ccd", bufs=3, space="DRAM"))

        neg1 = const.tile([128, 1], F32, tag="neg1", name="neg1")
        nc.vector.memset(neg1[:], -1.0)

        w2t = [const.tile([128, N_CELLS], BF16, tag=f"w2t{k}", name=f"w2t{k}")
               for k in range(KT)]
        for k in range(KT):
            nc.sync.dma_start(w2t[k][:], w2t_d[k * 128:(k + 1) * 128, :])

        # double-buffered transposed state s~ [cell(part), batch(free)], bf16
        st = [[const.tile([128, BSH], BF16, tag=f"st{p}{k}", name=f"st{p}{k}")
               for k in range(KT)] for p in range(2)]
        acc = [const.tile([128, BSH], F32, tag=f"acc{j}", name=f"acc{j}")
               for j in range(KT)]

        # ---------------- prologue: x^T arrives pre-transposed from the host
        for k in range(KT):
            nc.sync.dma_start(st[0][k][:], xt_d[k * 128:(k + 1) * 128, :])

        # local per-partition max of state_0 = x; launch AllReduce ASAP so the
        # first collective (incl. cross-core sync skew) hides under step 1.
        # (GpSimd has no free-dim reduce, so these stay on DVE in 2x bf16 mode)
        def emit_maxes(src_tiles, pmt_tile):
            for k in range(KT):
                scr = qp.tile([128, BSH], BF16, tag="q", name="q")
                nc.vector.tensor_scalar(scr[:], src_tiles[k][:], 1.0, None,
                                        op0=ALU.mult, op1=ALU.max,
                                        accum_out=pmt_tile[:, k:k + 1])

        pmt = small.tile([128, KT], F32, tag="pmt", name="pmt")
        emit_maxes(st[0], pmt)

        def launch_allreduce(pmt_tile):
            pm = small.tile([128, 1], F32, tag="pm", name="pm")
            nc.vector.reduce_max(pm[:], pmt_tile[:], axis=AX.X)
            pmr = small.tile([128, 1], F32, tag="pmr", name="pmr")
            nc.gpsimd.partition_all_reduce(pmr[:], pm[:], channels=128,
                                           reduce_op=bass_isa.ReduceOp.max)
            cin = small.tile([1, 8], F32, tag="cin", name="cin")
            nc.vector.memset(cin[:], 0.0)
            nc.vector.tensor_copy(cin[0:1, 0:1], pmr[0:1, 0:1])
            cc_in = ccd.tile([1, 8], F32, tag="ccin", name="ccin")
            cc_out = ccd.tile([1, 8], F32, tag="ccout", name="ccout")
            nc.gpsimd.dma_start(cc_in[:], cin[:])
            nc.gpsimd.collective_compute(
                "AllReduce", ALU.max, replica_groups=groups,
                ins=[cc_in.opt()], outs=[cc_out.opt()],
            )
            # gm readback on the Sync engine: its FIFO waits out the CC
            # latency so the Pool/DVE FIFOs never head-block on it
            gm = small.tile([1, 8], F32, tag="gm", name="gm")
            nc.sync.dma_start(gm[:], cc_out[:])
            return gm

        gm_pend = launch_allreduce(pmt)     # CC_0: G_0 = gmax(state_0)

        # acc init on ScalarE (idle during prologue): acc_j = w0 * x^T_j
        for j in range(KT):
            nc.scalar.mul(acc[j][:], st[0][j][:], float(w[0]))

        def consume_gm(gm, cvec_prev, t):
            """c_t = max(c_{t-1}*G_{t-1}, 0.1); coef_t = w_t*c_t.

            Emitted at the tail of iteration t's DVE block: CC_{t-1} has had
            a full step to complete, and the consumers (ACT/STT of t+1) give
            it another step of slack.
            """
            gmb = small.tile([128, 1], F32, tag="gmb", name="gmb")
            nc.gpsimd.partition_broadcast(gmb[:], gm[0:1, 0:1], channels=128)
            cvec = small.tile([128, 1], F32, tag="cvec", name="cvec", bufs=4)
            if cvec_prev is None:
                nc.vector.tensor_scalar(cvec[:], gmb[:], 0.1, None,
                                        op0=ALU.max)
            else:
                nc.vector.tensor_scalar(cvec[:], gmb[:], cvec_prev[:, 0:1],
                                        0.1, op0=ALU.mult, op1=ALU.max)
            coef = small.tile([128, 1], F32, tag="coef", name="coef", bufs=4)
            nc.vector.tensor_scalar(coef[:], cvec[:], float(w[t]), None,
                                    op0=ALU.mult)
            return cvec, coef

        cvec_prev = None                    # c_0 == 1.0 (imm scale at t=1)
        coef_prev = None                    # term 0 handled by acc init

        # ---------------- main steps
        for t in range(1, steps + 1):
            ph, prev = t % 2, (t - 1) % 2

            # delayed accumulate of term t-1 (2-step-slack collective data);
            # runs on DVE while this step's matmuls stream
            if coef_prev is not None:
                for j in range(KT):
                    nc.vector.scalar_tensor_tensor(
                        acc[j][:], st[prev][j][:], coef_prev[:, 0:1],
                        acc[j][:], op0=ALU.mult, op1=ALU.add,
                    )

            for j in range(KT):
                ps = psp.tile([128, BSH], F32, tag="ps", name="ps")
                for k in range(KT):
                    for b in range(NB):
                        nc.tensor.matmul(
                            ps[:, b * 512:(b + 1) * 512],
                            w2t[k][:, j * 128:(j + 1) * 128],
                            st[prev][k][:, b * 512:(b + 1) * 512],
                            start=(k == 0), stop=(k == KT - 1),
                        )
                nc.scalar.activation(
                    st[ph][j][:], ps[:], ACTF.Sigmoid,
                    bias=neg1[:, 0:1],
                    scale=(cvec_prev[:, 0:1] if cvec_prev is not None else 1.0),
                )

            if t < steps:
                pmt = small.tile([128, KT], F32, tag="pmt", name="pmt")
                emit_maxes(st[ph], pmt)
                gm_next = launch_allreduce(pmt)
            else:
                gm_next = None

            # consume CC_{t-1} at the FIFO tails
            cvec_prev, coef_prev = consume_gm(gm_pend, cvec_prev, t)
            gm_pend = gm_next

            if t == steps:
                # final term + output streaming
                for j in range(KT):
                    nc.vector.scalar_tensor_tensor(
                        acc[j][:], st[ph][j][:], coef_prev[:, 0:1], acc[j][:],
                        op0=ALU.mult, op1=ALU.add,
                    )
                    nc.sync.dma_start(out_d[j * 128:(j + 1) * 128, :],
                                      acc[j][:])

    nc.compile()
    return nc


def kernel(initial_activations, adjacency, std_devs, split_probs, join_probs,
           bounce_angles, step_weights, decay_rate, n_steps):
    global LAST_RESULTS
    x = np.ascontiguousarray(np.asarray(initial_activations, np.float32))
    steps, w2t_np, w = _host_prep(adjacency, std_devs, split_probs, join_probs,
                                  bounce_angles, step_weights, decay_rate,
                                  n_steps)
    if steps == 0:
        return (x * np.float32(1.0)).astype(np.float32)

    nc = _build_program(steps, w)

    bf16 = mybir.dt.np(BF16)
    w2tb = w2t_np.astype(np.float32).astype(bf16)
    in_maps = [
        {"xt": np.ascontiguousarray(x[c * BSH:(c + 1) * BSH].T).astype(bf16),
         "w2t": w2tb}
        for c in range(N_CORES)
    ]
    res = run_bass_kernel_spmd(
        nc, in_maps, core_ids=list(range(N_CORES)),
        trace=bool(os.environ.get("BASS_TRACE")),
    )
    LAST_RESULTS = res
    out = np.concatenate(
        [np.asarray(res.results[c]["out"], np.float32).T for c in range(N_CORES)],
        axis=0)
    return np.ascontiguousarray(out)


if __name__ == "__main__":
    rng = np.random.default_rng(0)
    ins = {
        "initial_activations": rng.random((BATCH, N_CELLS), np.float32),
        "adjacency": (rng.random((N_CELLS, N_CELLS)) < 6.0 / 512).astype(np.float32),
        "std_devs": rng.standard_normal(N_CELLS).astype(np.float32),
        "split_probs": rng.random(N_CELLS).astype(np.float32),
        "join_probs": rng.random(N_CELLS).astype(np.float32),
        "bounce_angles": (rng.random((N_CELLS, 6)) * 2).astype(np.float32),
        "step_weights": rng.standard_normal(10).astype(np.float32),
        "decay_rate": np.ones(1, np.float32),
        "n_steps": 9,
    }
    o = kernel(**ins)
    print("out", o.shape, o.dtype, float(o.mean()))
